# revision 1
# baseline (speedup 1.0000x reference)
"""Trainium2 Bass kernel for GNN edge-softmax attention message passing.

Strategy:
  - sort edges by destination row; 8 cores own contiguous 12544-row slices
  - 64-row blocks; per block, edges grouped by col-chunk (4 chunks of 25088
    cols so dma_gather's int16 indices reach the whole node table)
  - per 128-edge tile: gather [k|eigs|vhi|vlo|1] by col (1280B rows) and
    [q/sqrt(H)|eigs*exp(l0)] by local row (768B rows) with dma_gather,
    edge scores via DVE mul+reduce, exp on ACT, one-hot row matrices
    weighted by exp-scores on DVE, and a single bf16 matmul per tile
    accumulating [sum e0*v | d0] / [sum e1*v | d1] into PSUM
  - per-block raw [128,257] results go back to HBM; the final
    0.5*(P0/d0 + P1/d1) combine runs on host during unsharding
"""

import os
import sys
import types
import contextlib
import ctypes

import numpy as np

N = 100000
E = 3200000
H = 128
ED = 16
P6 = 6
NCORES = 8
R = 64                 # rows per block
CORE_ROWS = 12544      # 196 blocks of 64 rows
NPAD = CORE_ROWS * NCORES
NBLK = CORE_ROWS // R  # 196
NCHUNK = 4
CHUNK = NPAD // NCHUNK  # 25088
KVE_B = 1280           # bytes per kve row
QE_W = 192             # f32 per qe row (768B)
MAXCALL = 1024
LAST_EXEC_NS = None


def _install_axon_hooks():
    if "antenv.axon_hooks" in sys.modules:
        return
    mod = types.ModuleType("antenv.axon_hooks")
    _hook = [None]
    mod.set_axon_ntff_profile_hook = lambda h: _hook.__setitem__(0, h)
    mod.get_axon_ntff_profile_hook = lambda: _hook[0]
    sys.modules["antenv.axon_hooks"] = mod
    try:
        import antenv
        antenv.axon_hooks = mod
    except ImportError:
        pass
    try:
        from trn_agent_boot.trn_boot import _ntff_profile_via_ctypes
        h = _ntff_profile_via_ctypes("/opt/axon/libaxon_pjrt.so")
        if h is not None:
            mod.set_axon_ntff_profile_hook(h)
    except Exception:
        pass


def _prep(indices, path_type):
    """Sort/pad edges; returns per-core structures + the shared call plan."""
    row = indices[0].astype(np.int64)
    col = indices[1].astype(np.int64)
    core = row // CORE_ROWS
    blk = (row % CORE_ROWS) // R
    chunk = col // CHUNK
    # group key per edge: (core, blk, chunk)
    key = (core * NBLK + blk) * NCHUNK + chunk
    order = np.argsort(key, kind="stable")
    row_s, col_s, pt_s, key_s = row[order], col[order], path_type[order], key[order]
    ngroups = NCORES * NBLK * NCHUNK
    counts = np.bincount(key_s, minlength=ngroups).reshape(NCORES, NBLK, NCHUNK)
    # pad each (blk, chunk) group to a common multiple-of-128 size across cores
    gmax = counts.max(axis=0)  # [NBLK, NCHUNK]
    gpad = ((gmax + 127) // 128) * 128
    # split any group larger than MAXCALL into multiple calls
    calls = []  # list of (blk, chunk, n_idx) in processing order
    for b in range(NBLK):
        for c in range(NCHUNK):
            g = int(gpad[b, c])
            if g == 0:
                continue
            while g > 0:
                n = min(g, MAXCALL)
                calls.append((b, c, n))
                g -= n
    T = sum(n for _, _, n in calls) // 128  # tiles per core
    CW = sum(n // 16 for _, _, n in calls)

    # segment the call list at block boundaries so per-segment metadata
    # fits in SBUF; record (call_lo, call_hi, tile_lo, wcol_lo) per segment
    segs = []
    lo = 0
    tiles_acc = 0
    t_lo = 0
    w_lo = 0
    wcols_acc = 0
    for i, (b, c, n) in enumerate(calls):
        tiles_acc += n // 128
        wcols_acc += n // 16
        last_of_blk = (i + 1 == len(calls)) or calls[i + 1][0] != b
        if last_of_blk and (tiles_acc >= 384 or i + 1 == len(calls)):
            segs.append((lo, i + 1, t_lo, w_lo, tiles_acc, wcols_acc))
            lo = i + 1
            t_lo += tiles_acc
            w_lo += wcols_acc
            tiles_acc = 0
            wcols_acc = 0

    starts = np.zeros((NCORES, NBLK, NCHUNK), np.int64)
    flat = counts.reshape(NCORES, -1)
    np.cumsum(flat[:, :-1], axis=1, out=starts.reshape(NCORES, -1)[:, 1:])
    base = np.concatenate(([0], np.cumsum(np.bincount(core, minlength=NCORES))))

    per_core = []
    for cr in range(NCORES):
        cidx = np.zeros((128, CW), np.int16)
        ridx = np.zeros((128, CW), np.int16)
        roff = np.full((128, T), -1.0, np.float32)
        ptf = np.zeros((128, T), np.float32)
        wpos = 0
        tpos = 0
        for b in range(NBLK):
            for c in range(NCHUNK):
                gsz = int(gpad[b, c])
                if gsz == 0:
                    continue
                n_real = int(counts[cr, b, c])
                s = base[cr] + starts[cr, b, c]
                ccol = np.zeros(gsz, np.int64)
                crow = np.zeros(gsz, np.int64)   # local row in [0, CORE_ROWS)
                cro = np.full(gsz, -1.0, np.float32)
                cpt = np.zeros(gsz, np.float32)
                ccol[:n_real] = col_s[s:s + n_real] % CHUNK
                crow[:n_real] = row_s[s:s + n_real] % CORE_ROWS
                cro[:n_real] = (row_s[s:s + n_real] % CORE_ROWS) % R
                cpt[:n_real] = pt_s[s:s + n_real]
                # chop into calls of <= MAXCALL
                off = 0
                while off < gsz:
                    n = min(gsz - off, MAXCALL)
                    seg_c = ccol[off:off + n]
                    seg_r = crow[off:off + n]
                    cidx[:, wpos:wpos + n // 16] = np.tile(
                        seg_c.reshape(n // 16, 16).T.astype(np.int16), (8, 1))
                    ridx[:, wpos:wpos + n // 16] = np.tile(
                        seg_r.reshape(n // 16, 16).T.astype(np.int16), (8, 1))
                    nt = n // 128
                    roff[:, tpos:tpos + nt] = cro[off:off + n].reshape(nt, 128).T
                    ptf[:, tpos:tpos + nt] = cpt[off:off + n].reshape(nt, 128).T
                    wpos += n // 16
                    tpos += nt
                    off += n
        per_core.append(dict(cidx=cidx, ridx=ridx, roff=roff, ptf=ptf))
    return calls, T, CW, segs, per_core


def _build(calls, T, CW, segs, wvals):
    import concourse.mybir as mybir
    import concourse.tile as tile
    from concourse import bacc

    SEG_T = max(s[4] for s in segs)
    SEG_W = max(s[5] for s in segs)

    nc = bacc.Bacc(trn_type="TRN2", num_swdge_queues=4)
    kve = nc.dram_tensor("kve", [NPAD, KVE_B], mybir.dt.uint8, kind="ExternalInput")
    qe = nc.dram_tensor("qe", [CORE_ROWS, QE_W], mybir.dt.float32,
                        kind="ExternalInput")
    cidx = nc.dram_tensor("cidx", [128, CW], mybir.dt.int16, kind="ExternalInput")
    ridx = nc.dram_tensor("ridx", [128, CW], mybir.dt.int16, kind="ExternalInput")
    roff = nc.dram_tensor("roff", [128, T], mybir.dt.float32, kind="ExternalInput")
    ptf = nc.dram_tensor("ptf", [128, T], mybir.dt.float32, kind="ExternalInput")
    iota = nc.dram_tensor("iota", [128, R], mybir.dt.float32, kind="ExternalInput")
    raw = nc.dram_tensor("raw", [NBLK * 128, 257], mybir.dt.float32,
                         kind="ExternalOutput")

    with tile.TileContext(nc) as tc:
        with tc.tile_pool(name="const", bufs=1) as cpool, \
             tc.tile_pool(name="meta", bufs=2) as meta, \
             tc.tile_pool(name="gpool", bufs=4) as gpool, \
             tc.tile_pool(name="work", bufs=2) as work, \
             tc.tile_pool(name="psum", bufs=2, space="PSUM") as pp:
            iota_t = cpool.tile([128, R], mybir.dt.float32)
            nc.sync.dma_start(out=iota_t[:], in_=iota[:, :])

            for (clo, chi, t_lo, w_lo, nt_seg, nw_seg) in segs:
                cidx_t = meta.tile([128, SEG_W], mybir.dt.int16, tag="cidx")
                nc.sync.dma_start(out=cidx_t[:, :nw_seg],
                                  in_=cidx[:, w_lo:w_lo + nw_seg])
                ridx_t = meta.tile([128, SEG_W], mybir.dt.int16, tag="ridx")
                nc.sync.dma_start(out=ridx_t[:, :nw_seg],
                                  in_=ridx[:, w_lo:w_lo + nw_seg])
                roff_t = meta.tile([128, SEG_T], mybir.dt.float32, tag="roff")
                nc.sync.dma_start(out=roff_t[:, :nt_seg],
                                  in_=roff[:, t_lo:t_lo + nt_seg])
                ptf_t = meta.tile([128, SEG_T], mybir.dt.float32, tag="ptf")
                nc.sync.dma_start(out=ptf_t[:, :nt_seg],
                                  in_=ptf[:, t_lo:t_lo + nt_seg])

                # e1 = exp(path_emb_w)[path_type] via compare-mult ops
                e1_t = meta.tile([128, SEG_T], mybir.dt.float32, tag="e1")
                tmp_t = meta.tile([128, SEG_T], mybir.dt.float32, tag="tmp")
                acc_t = meta.tile([128, SEG_T], mybir.dt.float32, tag="acc")
                nc.vector.tensor_scalar(
                    out=e1_t[:, :nt_seg], in0=ptf_t[:, :nt_seg], scalar1=0.0,
                    scalar2=float(wvals[0]),
                    op0=mybir.AluOpType.is_equal, op1=mybir.AluOpType.mult)
                for p in range(1, P6):
                    nc.vector.tensor_scalar(
                        out=tmp_t[:, :nt_seg], in0=ptf_t[:, :nt_seg],
                        scalar1=float(p), scalar2=float(wvals[p]),
                        op0=mybir.AluOpType.is_equal, op1=mybir.AluOpType.mult)
                    src = e1_t if p % 2 == 1 else acc_t
                    dst = acc_t if p % 2 == 1 else e1_t
                    nc.vector.tensor_tensor(
                        out=dst[:, :nt_seg], in0=src[:, :nt_seg],
                        in1=tmp_t[:, :nt_seg], op=mybir.AluOpType.add)
                e1f = acc_t if (P6 - 1) % 2 == 1 else e1_t

                wpos = 0
                tpos = 0
                ps = None
                for ci in range(clo, chi):
                    b, c, n = calls[ci]
                    nt = n // 128
                    kg = gpool.tile([128, (MAXCALL // 128) * KVE_B],
                                    mybir.dt.uint8, tag="kg")
                    nc.gpsimd.dma_gather(
                        out_ap=kg[:, :nt * KVE_B].rearrange(
                            "p (n d) -> p n d", d=KVE_B),
                        in_ap=kve[c * CHUNK:(c + 1) * CHUNK, :],
                        idxs_ap=cidx_t[:, wpos:wpos + n // 16],
                        num_idxs=n, num_idxs_reg=n, elem_size=KVE_B,
                        queue_num=ci % 4)
                    qg = gpool.tile([128, (MAXCALL // 128) * QE_W],
                                    mybir.dt.float32, tag="qg")
                    nc.gpsimd.dma_gather(
                        out_ap=qg[:, :nt * QE_W].rearrange(
                            "p (n d) -> p n d", d=QE_W),
                        in_ap=qe[:],
                        idxs_ap=ridx_t[:, wpos:wpos + n // 16],
                        num_idxs=n, num_idxs_reg=n, elem_size=QE_W,
                        queue_num=(ci + 2) % 4)
                    wpos += n // 16

                    kf = kg[:].bitcast(mybir.dt.float32)
                    kb = kg[:].bitcast(mybir.dt.bfloat16)
                    prod = work.tile([128, (MAXCALL // 128) * 144],
                                     mybir.dt.float32, tag="prod")
                    qv = qg[:].rearrange("p (n d) -> p n d", d=QE_W)
                    kv = kf.rearrange("p (n d) -> p n d", d=KVE_B // 4)
                    nc.vector.tensor_tensor(
                        out=prod[:, :nt * 144].rearrange(
                            "p (n d) -> p n d", d=144),
                        in0=qv[:, :nt, 0:144], in1=kv[:, :nt, 0:144],
                        op=mybir.AluOpType.mult)
                    s0 = work.tile([128, MAXCALL // 128], mybir.dt.float32,
                                   tag="s0")
                    nc.vector.tensor_reduce(
                        out=s0[:, :nt],
                        in_=prod[:, :nt * 144].rearrange(
                            "p (n d) -> p n d", d=144),
                        axis=mybir.AxisListType.X, op=mybir.AluOpType.add)
                    e0 = work.tile([128, MAXCALL // 128], mybir.dt.float32,
                                   tag="e0")
                    nc.scalar.activation(
                        out=e0[:, :nt], in_=s0[:, :nt],
                        func=mybir.ActivationFunctionType.Exp)

                    A = work.tile([128, (MAXCALL // 128) * 2 * R],
                                  mybir.dt.bfloat16, tag="A")
                    oh = work.tile([128, (MAXCALL // 128) * R],
                                   mybir.dt.float32, tag="oh")
                    Av = A[:].rearrange("p (n d) -> p n d", d=2 * R)
                    ohv = oh[:].rearrange("p (n d) -> p n d", d=R)
                    nc.vector.tensor_tensor(
                        out=ohv[:, :nt, :],
                        in0=iota_t[:].rearrange("p (o d) -> p o d", o=1)
                            .to_broadcast([128, nt, R]),
                        in1=roff_t[:, tpos:tpos + nt].rearrange(
                            "p (n o) -> p n o", o=1).to_broadcast([128, nt, R]),
                        op=mybir.AluOpType.is_equal)
                    nc.vector.tensor_tensor(
                        out=Av[:, :nt, 0:R], in0=ohv[:, :nt, :],
                        in1=e0[:, :nt].rearrange("p (n o) -> p n o", o=1)
                            .to_broadcast([128, nt, R]),
                        op=mybir.AluOpType.mult)
                    nc.vector.tensor_tensor(
                        out=Av[:, :nt, R:2 * R], in0=ohv[:, :nt, :],
                        in1=e1f[:, tpos:tpos + nt].rearrange(
                            "p (n o) -> p n o", o=1).to_broadcast([128, nt, R]),
                        op=mybir.AluOpType.mult)

                    first = ps is None
                    if first:
                        ps = pp.tile([128, 257], mybir.dt.float32, tag="ps")
                    last_call_of_blk = (ci + 1 == len(calls)) or \
                        calls[ci + 1][0] != b
                    for j in range(nt):
                        rhs = kb[:, j * (KVE_B // 2) + 288:
                                 j * (KVE_B // 2) + 288 + 257]
                        nc.tensor.matmul(
                            out=ps[:], lhsT=A[:, j * 2 * R:(j + 1) * 2 * R],
                            rhs=rhs,
                            start=(first and j == 0),
                            stop=(last_call_of_blk and j == nt - 1))
                    tpos += nt

                    if last_call_of_blk:
                        ev = work.tile([128, 257], mybir.dt.float32, tag="ev")
                        nc.scalar.copy(out=ev[:], in_=ps[:])
                        nc.sync.dma_start(
                            out=raw[b * 128:(b + 1) * 128, :], in_=ev[:])
                        ps = None
    nc.finalize()
    return nc


def kernel(q, k, v, eigs, lambda0, path_emb_w, indices, path_type):
    _install_axon_hooks()
    q = np.asarray(q, np.float32)
    k = np.asarray(k, np.float32)
    v = np.asarray(v, np.float32)
    eigs = np.asarray(eigs, np.float32)
    lambda0 = np.asarray(lambda0, np.float32)
    path_emb_w = np.asarray(path_emb_w, np.float32)
    indices = np.asarray(indices, np.int32)
    path_type = np.asarray(path_type, np.int32)

    ew = float(np.exp(lambda0[0]))
    wvals = np.exp(path_emb_w[:, 0]).astype(np.float64)

    calls, T, CW, segs, per_core = _prep(indices, path_type)

    # kve table: [k f32 | eigs f32 | vhi bf16 | vlo bf16 | one bf16 | pad]
    kve = np.zeros((NPAD, KVE_B), np.uint8)
    kq = np.zeros((NPAD, 144), np.float32)
    kq[:N, :H] = k
    kq[:N, H:] = eigs
    kve[:, 0:576] = kq.view(np.uint8).reshape(NPAD, 576)
    from ml_dtypes import bfloat16
    vhi = v.astype(bfloat16)
    vlo = (v - vhi.astype(np.float32)).astype(bfloat16)
    kve[:N, 576:832] = vhi.view(np.uint8)
    kve[:N, 832:1088] = vlo.view(np.uint8)
    kve[:, 1088:1090] = np.tile(
        np.array([1.0], bfloat16).view(np.uint8), (NPAD, 1))

    qefull = np.zeros((NPAD, QE_W), np.float32)
    qefull[:N, :H] = q * (1.0 / np.sqrt(np.float32(H)))
    qefull[:N, H:144] = eigs * ew

    iota = np.tile(np.arange(R, dtype=np.float32), (128, 1))

    nc = _build(calls, T, CW, segs, wvals)

    in_maps = []
    for cr in range(NCORES):
        pc = per_core[cr]
        in_maps.append({
            "kve": kve,
            "qe": qefull[cr * CORE_ROWS:(cr + 1) * CORE_ROWS],
            "cidx": pc["cidx"], "ridx": pc["ridx"],
            "roff": pc["roff"], "ptf": pc["ptf"],
            "iota": iota,
        })

    from concourse.bass_utils import run_bass_kernel_spmd
    want_trace = bool(os.environ.get("KERNEL_TRACE"))
    res = run_bass_kernel_spmd(nc, in_maps, core_ids=list(range(NCORES)),
                               trace=want_trace)
    global LAST_EXEC_NS
    LAST_EXEC_NS = res.exec_time_ns

    out = np.zeros((NPAD, H), np.float32)
    for cr in range(NCORES):
        rawb = res.results[cr]["raw"].reshape(NBLK, 128, 257)
        p0 = rawb[:, 0:64, 0:128] + rawb[:, 0:64, 128:256]
        p1 = rawb[:, 64:128, 0:128] + rawb[:, 64:128, 128:256]
        d0 = rawb[:, 0:64, 256]
        d1 = rawb[:, 64:128, 256]
        d0 = np.where(d0 > 0, d0, 1.0)
        d1 = np.where(d1 > 0, d1, 1.0)
        blkout = 0.5 * (p0 / d0[..., None] + p1 / d1[..., None])
        out[cr * CORE_ROWS:(cr + 1) * CORE_ROWS] = blkout.reshape(CORE_ROWS, H)
    return out[:N]


if __name__ == "__main__":
    # small smoke test with synthetic inputs
    rng = np.random.default_rng(0)
    Et = int(os.environ.get("ET", "200000"))
    idx = rng.integers(0, N, size=(2, Et)).astype(np.int32)
    pt = rng.integers(0, P6, size=(Et,)).astype(np.int32)
    qq = rng.standard_normal((N, H), dtype=np.float32)
    kk = rng.standard_normal((N, H), dtype=np.float32)
    vv = rng.standard_normal((N, H), dtype=np.float32)
    ee = rng.standard_normal((N, ED), dtype=np.float32)
    l0 = np.zeros(1, np.float32)
    pw = rng.standard_normal((P6, 1), dtype=np.float32)

    out = kernel(qq, kk, vv, ee, l0, pw, idx, pt)

    # numpy reference
    row, col = idx[0], idx[1]
    x = (qq[row] * kk[col]).sum(-1) / np.sqrt(H) + np.exp(l0[0]) * (
        ee[row] * ee[col]).sum(-1)
    s1 = pw[pt, 0]
    exp0 = np.exp(x - x.max())
    d0 = np.zeros(N); np.add.at(d0, row, exp0)
    exp1 = np.exp(s1)
    d1 = np.zeros(N); np.add.at(d1, row, exp1)
    a = 0.5 * (exp0 / d0[row] + exp1 / d1[row])
    ref = np.zeros((N, H), np.float32)
    np.add.at(ref, row, a[:, None] * vv[col])
    num = np.linalg.norm(out - ref)
    den = np.linalg.norm(ref)
    print("rel err:", num / den)

